# revision 11
# baseline (speedup 1.0000x reference)
"""GroupedEmbedding lookup on 8 Trainium2 NeuronCores.

Problem: 8 tables [100000, 128] f32, 8 index vectors [200000] int64.
Output: per-table gather concatenated -> [1600000, 128] f32.

Sharding: table-parallel. Core c holds table c and its 200000 indices;
it gathers locally. No collectives. Host concatenates the 8 slices.

Per-core kernel (v4):
  Output rows are assigned partition-major: partition p owns output rows
  p*TILES_PAD .. p*TILES_PAD+TILES_PAD-1, so the index upload is a plain
  reshape of values to [128, TILES_PAD] and the output tensor
  [128, TILES_PAD, 128] flattens straight back to row order.

  Gathers use the proven walrus indirect-DMA contract: offset AP [128,1],
  dest [128, dim] - one embedding row per partition per instruction (this
  ucode build caps indirect DMA at 128 rows/instruction; the stream runs
  at a fixed ~1.41us/instruction pace, which is the kernel's floor).
  TILES_PAD=1564 is the minimal group-divisible tile count. The index
  upload is split: gpsimd loads only group 0's columns (fast start) while
  the sync engine loads the rest concurrently. Partition p's slice of a
  68-tile group is contiguous in DRAM (34.8KB), so each store DMA is 128
  large descriptors. Double-buffered; raw Bass semaphores (this walrus
  build encodes at most one sync wait per DMA, so waits are standalone
  wait_ge ops and DMAs carry only sem updates).
"""

import os
import sys

for _p in ("/root/.axon_site", "/root/.axon_site/_ro/trn_rl_repo",
           "/root/.axon_site/_ro/pypackages", "/opt/trn_rl_repo"):
    if os.path.isdir(_p) and _p not in sys.path:
        sys.path.append(_p)

from contextlib import ExitStack

import numpy as np

import concourse.bass as bass
import concourse.mybir as mybir
from concourse.bass_utils import run_bass_kernel_spmd

NUM_TABLES = 8
NUM_EMBEDDINGS = 100000
EMBED_DIM = 128
IDS_PER_FEATURE = 200000

P = 128
TILES_PAD = 1564             # 200000 ids -> 1562.5 tiles, pad to 23*68
ROWS_PAD = TILES_PAD * P     # 200192
GROUP = 68                   # tiles per store group; 1564 = 23*68


def build_nc(num_emb=NUM_EMBEDDINGS, tiles_pad=TILES_PAD, dim=EMBED_DIM,
             group=GROUP):
    """Build the per-core Bass program (SPMD: same program on all cores)."""
    assert tiles_pad % group == 0
    n_groups = tiles_pad // group
    nc = bass.Bass()
    idx = nc.dram_tensor("idx", [P, tiles_pad], mybir.dt.int32,
                         kind="ExternalInput")
    table = nc.dram_tensor("table", [num_emb, dim], mybir.dt.float32,
                           kind="ExternalInput")
    # out[p][t][d] = row p*tiles_pad+t of the padded output.
    out = nc.dram_tensor("out", [P, tiles_pad, dim], mybir.dt.float32,
                         kind="ExternalOutput")

    with ExitStack() as es:
        block = es.enter_context(nc.Block())
        idx_semA = es.enter_context(nc.semaphore("idx_semA"))
        idx_semB = es.enter_context(nc.semaphore("idx_semB"))
        # Parity-split semaphores: only one group's DMAs are ever in
        # flight per sem, so cumulative waits are race-free.
        g_sems = [es.enter_context(nc.semaphore(f"g_sem{b}")) for b in (0, 1)]
        w_sems = [es.enter_context(nc.semaphore(f"w_sem{b}")) for b in (0, 1)]
        # Quarter sems for the split tail store: a sub-store may only wait
        # on a FULL batch count of its own sem (per-engine completion
        # interleaving makes intermediate counts racy).
        q_sems = [es.enter_context(nc.semaphore(f"q_sem{q}"))
                  for q in range(4)]
        idx_sb = es.enter_context(
            nc.sbuf_tensor("idx_sb", [P, tiles_pad], mybir.dt.int32))
        gbuf = es.enter_context(
            nc.sbuf_tensor("gbuf", [P, 2 * group * dim], mybir.dt.float32))

        @block.gpsimd
        def _(gp):
            # Fast path: only group 0's index columns block the first gather.
            gp.dma_start(out=idx_sb[:, :group],
                         in_=idx[:, :group]).then_inc(idx_semA, 16)
            gp.wait_ge(idx_semA, 16)
            for k in range(n_groups):
                b = k % 2
                if k == 1:
                    gp.wait_ge(idx_semB, 16)
                if k >= 2:
                    gp.wait_ge(w_sems[b], 16 * (k // 2))
                last_split = k == n_groups - 1 and group % 4 == 0
                for j in range(group):
                    t = k * group + j
                    o = (b * group + j) * dim
                    inst = gp.indirect_dma_start(
                        out=gbuf[:, o:o + dim],
                        out_offset=None,
                        in_=table[:, :],
                        in_offset=bass.IndirectOffsetOnAxis(
                            ap=idx_sb[:, t:t + 1], axis=0),
                    )
                    if last_split:
                        inst.then_inc(q_sems[j // (group // 4)], 16)
                    else:
                        inst.then_inc(g_sems[b], 16)

        @block.sync
        def _(sy):
            # Bulk of the index upload overlaps with group 0's gathers.
            sy.dma_start(out=idx_sb[:, group:],
                         in_=idx[:, group:]).then_inc(idx_semB, 16)
            for k in range(n_groups):
                b = k % 2
                o = b * group * dim
                if k == n_groups - 1 and group % 4 == 0:
                    # Split the final store in quarters: three drain while
                    # the last gathers are still in flight, leaving only a
                    # quarter-store on the critical-path tail.
                    sub = group // 4
                    for q in range(4):
                        lo, hi = sub * q, sub * (q + 1)
                        sy.wait_ge(q_sems[q], 16 * sub)
                        sy.dma_start(
                            out=out[:, k * group + lo:k * group + hi, :],
                            in_=gbuf[:, o + lo * dim:o + hi * dim],
                        ).then_inc(w_sems[b], 16)
                else:
                    sy.wait_ge(g_sems[b], 16 * group * (k // 2 + 1))
                    sy.dma_start(
                        out=out[:, k * group:(k + 1) * group, :],
                        in_=gbuf[:, o:o + group * dim],
                    ).then_inc(w_sems[b], 16)
    return nc


_NC_CACHE = {}


def _get_nc():
    key = "full"
    if key not in _NC_CACHE:
        _NC_CACHE[key] = build_nc()
    return _NC_CACHE[key]


def run(values: np.ndarray, weights: np.ndarray, trace: bool = False, **kw):
    assert values.shape == (NUM_TABLES, IDS_PER_FEATURE)
    assert weights.shape == (NUM_TABLES, NUM_EMBEDDINGS, EMBED_DIM)

    nc = _get_nc()

    idx_pad = np.zeros((NUM_TABLES, ROWS_PAD), dtype=np.int32)
    idx_pad[:, :IDS_PER_FEATURE] = values.astype(np.int32)
    # partition-major: idxT[c][p][t] = idx of output row p*TILES_PAD+t
    idx_t = idx_pad.reshape(NUM_TABLES, P, TILES_PAD)

    w = np.ascontiguousarray(weights, dtype=np.float32)
    in_maps = [{"idx": idx_t[c], "table": w[c]} for c in range(NUM_TABLES)]
    res = run_bass_kernel_spmd(nc, in_maps, core_ids=list(range(NUM_TABLES)),
                               trace=trace, **kw)
    outs = [
        r["out"].reshape(ROWS_PAD, EMBED_DIM)[:IDS_PER_FEATURE]
        for r in res.results
    ]
    return np.concatenate(outs, axis=0), res


def kernel(values: np.ndarray, weights: np.ndarray) -> np.ndarray:
    return run(values, weights)[0]


# revision 13
# speedup vs baseline: 1.0062x; 1.0062x over previous
"""GroupedEmbedding lookup on 8 Trainium2 NeuronCores.

Problem: 8 tables [100000, 128] f32, 8 index vectors [200000] int64.
Output: per-table gather concatenated -> [1600000, 128] f32.

Sharding: table-parallel. Core c holds table c and its 200000 indices;
it gathers locally. No collectives. Host concatenates the 8 slices.

Per-core kernel (v4):
  Output rows are assigned partition-major: partition p owns output rows
  p*TILES_PAD .. p*TILES_PAD+TILES_PAD-1, so the index upload is a plain
  reshape of values to [128, TILES_PAD] and the output tensor
  [128, TILES_PAD, 128] flattens straight back to row order.

  Gathers use the proven walrus indirect-DMA contract: offset AP [128,1],
  dest [128, dim] - one embedding row per partition per instruction (this
  ucode build caps indirect DMA at 128 rows/instruction; the stream runs
  at a fixed ~1.41us/instruction pace, which is the kernel's floor).
  TILES_PAD=1564 is the minimal group-divisible tile count. The index
  upload is split: gpsimd loads only group 0's columns (fast start) while
  the sync engine loads the rest concurrently. Partition p's slice of a
  68-tile group is contiguous in DRAM (34.8KB), so each store DMA is 128
  large descriptors. Double-buffered; raw Bass semaphores (this walrus
  build encodes at most one sync wait per DMA, so waits are standalone
  wait_ge ops and DMAs carry only sem updates).
"""

import os
import sys

for _p in ("/root/.axon_site", "/root/.axon_site/_ro/trn_rl_repo",
           "/root/.axon_site/_ro/pypackages", "/opt/trn_rl_repo"):
    if os.path.isdir(_p) and _p not in sys.path:
        sys.path.append(_p)

from contextlib import ExitStack

import numpy as np

import concourse.bass as bass
import concourse.mybir as mybir
from concourse.bass_utils import run_bass_kernel_spmd

NUM_TABLES = 8
NUM_EMBEDDINGS = 100000
EMBED_DIM = 128
IDS_PER_FEATURE = 200000

P = 128
TILES_PAD = 1564             # 200000 ids -> 1562.5 tiles, pad to 23*68
ROWS_PAD = TILES_PAD * P     # 200192
GROUP = 68                   # tiles per store group; 1564 = 23*68


def build_nc(num_emb=NUM_EMBEDDINGS, tiles_pad=TILES_PAD, dim=EMBED_DIM,
             group=GROUP, sizes=None):
    """Build the per-core Bass program (SPMD: same program on all cores)."""
    if sizes is None:
        assert tiles_pad % group == 0
        sizes = [group] * (tiles_pad // group)
    assert sum(sizes) == tiles_pad and max(sizes) <= group
    starts = [sum(sizes[:k]) for k in range(len(sizes))]
    n_groups = len(sizes)
    nc = bass.Bass()
    idx = nc.dram_tensor("idx", [P, tiles_pad], mybir.dt.int32,
                         kind="ExternalInput")
    table = nc.dram_tensor("table", [num_emb, dim], mybir.dt.float32,
                           kind="ExternalInput")
    # out[p][t][d] = row p*tiles_pad+t of the padded output.
    out = nc.dram_tensor("out", [P, tiles_pad, dim], mybir.dt.float32,
                         kind="ExternalOutput")

    with ExitStack() as es:
        block = es.enter_context(nc.Block())
        idx_semA = es.enter_context(nc.semaphore("idx_semA"))
        idx_semB = es.enter_context(nc.semaphore("idx_semB"))
        # Parity-split semaphores: only one group's DMAs are ever in
        # flight per sem, so cumulative waits are race-free.
        g_sems = [es.enter_context(nc.semaphore(f"g_sem{b}")) for b in (0, 1)]
        w_sems = [es.enter_context(nc.semaphore(f"w_sem{b}")) for b in (0, 1)]
        idx_sb = es.enter_context(
            nc.sbuf_tensor("idx_sb", [P, tiles_pad], mybir.dt.int32))
        gbuf = es.enter_context(
            nc.sbuf_tensor("gbuf", [P, 2 * group * dim], mybir.dt.float32))

        @block.gpsimd
        def _(gp):
            # Fast path: only group 0's index columns block the first gather.
            gp.dma_start(out=idx_sb[:, :sizes[0]],
                         in_=idx[:, :sizes[0]]).then_inc(idx_semA, 16)
            gp.wait_ge(idx_semA, 16)
            for k in range(n_groups):
                b = k % 2
                if k == 1:
                    gp.wait_ge(idx_semB, 16)
                if k >= 2:
                    gp.wait_ge(w_sems[b], 16 * (k // 2))
                for j in range(sizes[k]):
                    t = starts[k] + j
                    o = (b * group + j) * dim
                    gp.indirect_dma_start(
                        out=gbuf[:, o:o + dim],
                        out_offset=None,
                        in_=table[:, :],
                        in_offset=bass.IndirectOffsetOnAxis(
                            ap=idx_sb[:, t:t + 1], axis=0),
                    ).then_inc(g_sems[b], 16)

        @block.sync
        def _(sy):
            # Bulk of the index upload overlaps with group 0's gathers.
            sy.dma_start(out=idx_sb[:, sizes[0]:],
                         in_=idx[:, sizes[0]:]).then_inc(idx_semB, 16)
            for k in range(n_groups):
                b = k % 2
                done = sum(sizes[j] for j in range(k + 1) if j % 2 == b)
                sy.wait_ge(g_sems[b], 16 * done)
                o = b * group * dim
                sy.dma_start(
                    out=out[:, starts[k]:starts[k] + sizes[k], :],
                    in_=gbuf[:, o:o + sizes[k] * dim],
                ).then_inc(w_sems[b], 16)
    return nc


_NC_CACHE = {}


def _get_nc():
    key = "full"
    if key not in _NC_CACHE:
        # 22 full groups + 4 short tail groups: the final store that sits
        # entirely after the last gather shrinks from 68 to 17 tiles
        # (13.5us -> 3.4us of critical-path tail).
        _NC_CACHE[key] = build_nc(sizes=[68] * 22 + [17] * 4)
    return _NC_CACHE[key]


def run(values: np.ndarray, weights: np.ndarray, trace: bool = False, **kw):
    assert values.shape == (NUM_TABLES, IDS_PER_FEATURE)
    assert weights.shape == (NUM_TABLES, NUM_EMBEDDINGS, EMBED_DIM)

    nc = _get_nc()

    idx_pad = np.zeros((NUM_TABLES, ROWS_PAD), dtype=np.int32)
    idx_pad[:, :IDS_PER_FEATURE] = values.astype(np.int32)
    # partition-major: idxT[c][p][t] = idx of output row p*TILES_PAD+t
    idx_t = idx_pad.reshape(NUM_TABLES, P, TILES_PAD)

    w = np.ascontiguousarray(weights, dtype=np.float32)
    in_maps = [{"idx": idx_t[c], "table": w[c]} for c in range(NUM_TABLES)]
    res = run_bass_kernel_spmd(nc, in_maps, core_ids=list(range(NUM_TABLES)),
                               trace=trace, **kw)
    outs = [
        r["out"].reshape(ROWS_PAD, EMBED_DIM)[:IDS_PER_FEATURE]
        for r in res.results
    ]
    return np.concatenate(outs, axis=0), res


def kernel(values: np.ndarray, weights: np.ndarray) -> np.ndarray:
    return run(values, weights)[0]
